# revision 42
# baseline (speedup 1.0000x reference)
"""Cross-attention kernel for Trainium2, 8 NeuronCores.

Sharding: batch (4) x head-group (2) = 8 cores; host sums the two head-group
partial out-projections per batch and adds the bias.

Per-core design (v3, all-bf16 numerics):
- Per (ktile, head-pair) the two heads' 64-row score matmuls sit on disjoint
  PE row halves (tile_position via base partition) and run concurrently
  (~1.45x measured vs serial half-array).
- Softmax exp alternates by ktile between ScalarE (exact, 3/4 of tiles) and
  a custom 8-stage DVE op computing (1 + s/64)^64 (1/4 of tiles); each pt
  tile is written entirely by one engine so no cross-engine write overlap
  (hull-based dependency tracking serializes otherwise).
- PV: bf16 [128,65] V-stationary (col 64 = ones -> softmax denominator in
  PSUM row 64) x [128,512] prob-stream.
- Normalization: DVE drains acc, DMA partition-shifts the denominator row,
  DVE reciprocal, GpSimd broadcast + multiply (GpSimd cannot read PSUM);
  odd head's rows reach atp via a DMA partition shift.
- QKV/O projections (bf16, 8 accumulating matmuls per 512-chunk, FWL) drain
  from a background queue between attention steps so the PE never idles.
- PSUM: 2x scores [128,1024] (4 banks) + 2 accumulators (2) + proj (2).
"""

import numpy as np
import ml_dtypes

import concourse.bacc as bacc
import concourse.mybir as mybir
import concourse.tile as tile
from concourse.bass import dve_ver_for
from concourse.bass_utils import run_bass_kernel_spmd
from concourse import dve_ops
from concourse.dve_spec import Spec, Src0, C0, One, sq, lower, _has_src1
from concourse.dve_uop import DveOpSpec

BF16 = mybir.dt.bfloat16
F32 = mybir.dt.float32

B, S, D = 4, 2048, 1024
H_TOT, DH = 16, 64
H = 8                      # heads per core
DG = H * DH                # 512, head-group width
N_CORES = 8
P = 128
NK = 16                    # 128-row kpos tiles
NCH = 4                    # q chunks of 512
CH = 512
# exp engine per ktile: True -> ScalarE (exact), False -> VectorE (approx);
# k=0 goes to VectorE: phase-start steps are long anyway (acc WAR + norm)
EXP_PAT = [False] + [True] * 15

_CACHED_NC = None
LAST_RESULT = None


def _ref_exp64(in0, in1, c0, c1, c2):
    y = 1.0 + in0 * c0
    for _ in range(6):
        y = y * y
    return y


def _register_exp_op():
    for op in dve_ops.OPS:
        if op.name == "EXP_APPROX64":
            return op
    body = One + Src0 * C0
    for _ in range(6):
        body = sq(body)
    spec = Spec(body=body, reference=_ref_exp64)
    ver = dve_ver_for("TRN2")
    row = dve_ops._CUSTOM_DVE_ROW_BASE + len(dve_ops.OPS)
    tmp = DveOpSpec(name="EXP_APPROX64", opcode=row,
                    uops=lower(spec, ver=ver), rd1_en=_has_src1(spec))
    op = dve_ops.DveOp("EXP_APPROX64", spec, subdim=False,
                       uops_sha={ver: tmp.sha(ver)})
    dve_ops.OPS.append(op)
    dve_ops.CUSTOM_DVE_SPECS[op.name] = op.spec
    dve_ops._SUB_OPCODE_FOR_NAME[op.name] = row
    return op


EXP_OP = _register_exp_op()
Exp = mybir.ActivationFunctionType.Exp


def _emit_kernel():
    nc = bacc.Bacc()
    xt = nc.dram_tensor("xt", [P, 8, S], BF16, kind="ExternalInput")
    ct = nc.dram_tensor("ct", [P, 8, S], BF16, kind="ExternalInput")
    wq = nc.dram_tensor("wq", [P, 4, 8, P], BF16, kind="ExternalInput")
    wk = nc.dram_tensor("wk", [P, 4, 8, P], BF16, kind="ExternalInput")
    wv = nc.dram_tensor("wv", [P, 8, DG], BF16, kind="ExternalInput")
    wo = nc.dram_tensor("wo", [P, 4, D], BF16, kind="ExternalInput")
    outT = nc.dram_tensor("outT", [D, S], BF16, kind="ExternalOutput")

    with tile.TileContext(nc) as tc:
        with tc.tile_pool(name="big", bufs=1) as big, \
             tc.tile_pool(name="ptp", bufs=1) as ptp, \
             tc.tile_pool(name="nrm", bufs=1) as nrm, \
             tc.tile_pool(name="osg", bufs=3) as osg, \
             tc.tile_pool(name="ps", bufs=1, space="PSUM") as ps:

            xt_sb = big.tile([P, 8, S], BF16, tag="xt")
            ct_sb = big.tile([P, 8, S], BF16, tag="ct")
            wq_sb = big.tile([P, 4, 8, P], BF16, tag="wq")
            wk_sb = big.tile([P, 4, 8, P], BF16, tag="wk")
            wv_sb = big.tile([P, 8, DG], BF16, tag="wv")
            wo_sb = big.tile([P, 4, D], BF16, tag="wo")
            qt = [big.tile([P, S], BF16, tag=f"qt{m}", name=f"qt{m}")
                  for m in range(4)]
            kt = [big.tile([P, S], BF16, tag=f"kt{m}", name=f"kt{m}")
                  for m in range(4)]
            # V stationary [(ktile*8+head), 66]: cols 0:64 V, col 64 ones
            vt = big.tile([P, NK * H, 66], BF16, tag="vt")
            atp = [[big.tile([P, CH], BF16, tag=f"at{c}{m}", name=f"at{c}{m}")
                    for m in range(4)] for c in range(NCH)]

            ones_f = big.tile([P, 64], BF16, tag="ones")
            nc.vector.memset(vt[:, :, 64:65], 1.0)
            nc.vector.memset(ones_f[:], 1.0)

            # input DMAs, arrival-ordered for the prelude; weight tensors
            # split by head-pair so kproj/qproj(0,0) start early
            nc.sync.dma_start(ct_sb[:, :, 0:512], ct[:, :, 0:512])
            nc.sync.dma_start(wk_sb[:, 0], wk[:, 0])
            nc.sync.dma_start(xt_sb[:, :, 0:512], xt[:, :, 0:512])
            nc.sync.dma_start(wq_sb[:, 0], wq[:, 0])
            nc.sync.dma_start(wv_sb[:], wv[:])
            nc.sync.dma_start(wk_sb[:, 1:4], wk[:, 1:4])
            nc.sync.dma_start(wq_sb[:, 1:4], wq[:, 1:4])
            for s4 in range(1, 4):
                sl = slice(s4 * 512, (s4 + 1) * 512)
                nc.sync.dma_start(ct_sb[:, :, sl], ct[:, :, sl])
            nc.sync.dma_start(wo_sb[:], wo[:])
            for s4 in range(1, 4):
                sl = slice(s4 * 512, (s4 + 1) * 512)
                nc.sync.dma_start(xt_sb[:, :, sl], xt[:, :, sl])

            # ---- projection emitters (bf16, 8 accumulating MMs) ----
            def kproj(m, s4):
                pj = ps.tile([P, 512], F32, tag="pj", bufs=2)
                for a in range(8):
                    nc.tensor.matmul(
                        pj[:], wk_sb[:, m, a, :],
                        ct_sb[:, a, s4 * 512:(s4 + 1) * 512],
                        start=(a == 0), stop=(a == 7))
                nc.vector.tensor_copy(kt[m][:, s4 * 512:(s4 + 1) * 512], pj[:])

            def qproj(m, qc):
                pj = ps.tile([P, 512], F32, tag="pj", bufs=2)
                for a in range(8):
                    nc.tensor.matmul(
                        pj[:], wq_sb[:, m, a, :],
                        xt_sb[:, a, qc * 512:(qc + 1) * 512],
                        start=(a == 0), stop=(a == 7))
                nc.vector.tensor_copy(qt[m][:, qc * 512:(qc + 1) * 512], pj[:])

            def vproj(k):
                pj = ps.tile([P, 512], F32, tag="pj", bufs=2)
                for a in range(8):
                    nc.tensor.matmul(
                        pj[:], ct_sb[:, a, k * P:(k + 1) * P],
                        wv_sb[:, a, :],
                        start=(a == 0), stop=(a == 7))
                nc.vector.tensor_copy(
                    vt[:, k * H:(k + 1) * H, 0:64],
                    pj[:].rearrange("p (h d) -> p h d", h=H))

            def oproj(mt, c):
                pj = ps.tile([P, 512], F32, tag="pj", bufs=2)
                for m in range(4):
                    nc.tensor.matmul(
                        pj[:], wo_sb[:, m, mt * P:(mt + 1) * P],
                        atp[c][m][:], start=(m == 0), stop=(m == 3))
                osl = osg.tile([P, 512], BF16, tag="osl")
                nc.vector.tensor_copy(osl[:], pj[:])
                nc.sync.dma_start(
                    outT[mt * P:(mt + 1) * P, c * 512:(c + 1) * 512], osl[:])

            # background queue: (release_step, seq, emit_fn), kept sorted
            import bisect
            bg = []
            bg_seq = [0]

            def bg_add(due, fn):
                bg_seq[0] += 1
                bisect.insort(bg, (due, bg_seq[0], fn))

            def drain_tick(g):
                while bg and bg[0][0] <= g:
                    bg.pop(0)[2]()

            def drain_all():
                while bg:
                    bg.pop(0)[2]()

            # ---- prelude: minimum to start (c0, m0) ----
            kproj(0, 0)
            qproj(0, 0)
            vproj(0)

            # kt[m] seq-chunk s4 feeds ktiles 4*s4.. of phase (c, m, *)
            for m in range(4):
                for s4 in range(4):
                    if m == 0 and s4 == 0:
                        continue
                    need = m * 16 + 4 * s4
                    bg_add(max(0, need - 3), lambda m=m, s4=s4: kproj(m, s4))
            # qt[m] chunk qc feeds phase (qc, m, *)
            for m in range(4):
                for qc in range(4):
                    if m == 0 and qc == 0:
                        continue
                    need = qc * 64 + m * 16
                    bg_add(max(0, need - 3), lambda m=m, qc=qc: qproj(m, qc))
            # vt ktile k first used at step k of (c0, m0)
            for k in range(1, NK):
                bg_add(max(0, k - 2), lambda k=k: vproj(k))

            def norm_pair(c, m, acc_a, acc_b):
                # drain accs to SBUF (DVE); broadcast each denominator row
                # across 64 partitions with a K=1 ones-matmul on TensorE
                # (engines can't cross partitions; DMA/GpSimd round-trips
                # head-of-line block their queues); DVE reciprocal + multiply.
                # Odd head's rows reach atp via a leaf DMA partition shift.
                ua = nrm.tile([65, CH], BF16, tag="usba", bufs=2, name="usba")
                ub = nrm.tile([65, CH], BF16, tag="usbb", bufs=2, name="usbb")
                ba = nrm.tile([64, CH], F32, tag="bca", bufs=2, name="bca")
                bb = nrm.tile([64, CH], F32, tag="bcb", bufs=2, name="bcb")
                stg = nrm.tile([64, CH], BF16, tag="stg", bufs=2, name="stg")
                nc.vector.tensor_copy(ua[:], acc_a[0:65, :])
                nc.vector.tensor_copy(ub[:], acc_b[0:65, :])
                za = ps.tile([P, CH], F32, tag="pj", bufs=2, name="za")
                nc.tensor.matmul(za[0:64, :], ones_f[64:65, 0:64],
                                 ua[64:65, :], start=True, stop=True)
                nc.vector.reciprocal_approx_fast(ba[:], za[0:64, :])
                zb = ps.tile([P, CH], F32, tag="pj", bufs=2, name="zb")
                nc.tensor.matmul(zb[0:64, :], ones_f[64:65, 0:64],
                                 ub[64:65, :], start=True, stop=True)
                nc.vector.reciprocal_approx_fast(bb[:], zb[0:64, :])
                nc.vector.tensor_mul(atp[c][m][0:64, :], ua[0:64, :], ba[:])
                nc.vector.tensor_mul(stg[:], ub[0:64, :], bb[:])
                nc.sync.dma_start(atp[c][m][64:128, :], stg[:])

            # ---- flat step loop over (chunk, head-pair, ktile) ----
            pv_q = []
            nrm_q = []

            def tick_queues(g):
                while pv_q and pv_q[0][0] <= g:
                    pv_q.pop(0)[1]()
                while nrm_q and nrm_q[0][0] <= g:
                    nrm_q.pop(0)[1]()

            accs = {}
            for g in range(256):
                c, m, k = g // 64, (g // 16) % 4, g % 16
                if k == 0:
                    accs[(m, 0)] = ps.tile([P, CH], F32, tag="acc",
                                           bufs=2, name="accA")
                    accs[(m, 1)] = ps.tile([P, CH], F32, tag="acc",
                                           bufs=2, name="accB")
                # scores: both heads on disjoint PE row halves
                scb = ps.tile([P, 1024], F32, tag="sc", bufs=2, name="sc")
                nc.tensor.matmul(scb[:, 0:512],
                                 kt[m][0:64, k * P:(k + 1) * P],
                                 qt[m][0:64, c * CH:(c + 1) * CH],
                                 start=True, stop=True)
                nc.tensor.matmul(scb[:, 512:1024],
                                 kt[m][64:128, k * P:(k + 1) * P],
                                 qt[m][64:128, c * CH:(c + 1) * CH],
                                 start=True, stop=True)
                tick_queues(g)
                drain_tick(g)
                # exp on one engine per ktile, bf16 probs out
                pt = ptp.tile([P, 2, CH], BF16, tag="pt", bufs=4, name="pt")
                scv = scb[:].rearrange("p (h q) -> p h q", h=2)
                if EXP_PAT[k]:
                    nc.scalar.activation(pt[:], scv[:], Exp)
                else:
                    nc.vector._custom_dve(EXP_OP, out=pt[:], in0=scv[:],
                                          s0=1.0 / 64.0)

                def pv(c=c, m=m, k=k, pt=pt):
                    for hh in range(2):
                        nc.tensor.matmul(
                            accs[(m, hh)][0:65, :],
                            vt[:, k * H + 2 * m + hh, 0:65],
                            pt[:, hh, :],
                            start=(k == 0), stop=(k == NK - 1))
                    if k == NK - 1:
                        nrm_q.append(
                            (0, lambda c=c, m=m,
                                a=accs.pop((m, 0)), b=accs.pop((m, 1)):
                                norm_pair(c, m, a, b)))
                        if m == 3:
                            for mt in range(8):
                                bg_add(g + 35 + 2 * mt,
                                       lambda mt=mt, c=c: oproj(mt, c))
                pv_q.append((g + 3, pv))

            while pv_q:
                pv_q.pop(0)[1]()
            while nrm_q:
                nrm_q.pop(0)[1]()
            drain_all()

    nc.compile()
    return nc


def _get_nc():
    global _CACHED_NC
    if _CACHED_NC is None:
        _CACHED_NC = _emit_kernel()
    return _CACHED_NC


def _xpose_layout(xT):
    # xT: [D, S] f32 -> [128, 8, S] bf16 with d = a*128 + p
    a = xT.reshape(8, P, xT.shape[1])
    return np.ascontiguousarray(a.transpose(1, 0, 2)).astype(
        ml_dtypes.bfloat16)


def kernel(inputs, context, Wq, Wk, Wv, Wo, bo, **kw):
    global LAST_RESULT
    scale = DH ** -0.5
    wq_s = np.asarray(Wq, np.float32) * scale
    wk_s = np.asarray(Wk, np.float32)
    wv_s = np.asarray(Wv, np.float32)
    wo_s = np.asarray(Wo, np.float32)

    xt = [_xpose_layout(np.asarray(inputs[b], np.float32).T)
          for b in range(B)]
    ct = [_xpose_layout(np.asarray(context[b], np.float32).T)
          for b in range(B)]
    def _wm_layout(w):
        # w: [D, DG] f32 -> [128, 4m, 8a, 128] bf16, (p,m,a,n) = w[a*128+p,
        # m*128+n]; m-slices contiguous so the prelude DMA is small
        a = w.reshape(8, P, 4, P)
        return np.ascontiguousarray(a.transpose(1, 2, 0, 3)).astype(
            ml_dtypes.bfloat16)

    wq8 = [_wm_layout(wq_s[:, gg * DG:(gg + 1) * DG]) for gg in range(2)]
    wk8 = [_wm_layout(wk_s[:, gg * DG:(gg + 1) * DG]) for gg in range(2)]
    wv8 = [_xpose_layout(wv_s[:, gg * DG:(gg + 1) * DG]) for gg in range(2)]
    wo8 = []
    for gg in range(2):
        a = wo_s[gg * DG:(gg + 1) * DG, :].reshape(4, P, D)
        wo8.append(np.ascontiguousarray(a.transpose(1, 0, 2)).astype(
            ml_dtypes.bfloat16))

    in_maps = []
    for core in range(N_CORES):
        b, gg = core // 2, core % 2
        in_maps.append({
            "xt": xt[b], "ct": ct[b],
            "wq": wq8[gg], "wk": wk8[gg], "wv": wv8[gg],
            "wo": wo8[gg],
        })

    nc = _get_nc()
    res = run_bass_kernel_spmd(nc, in_maps, core_ids=list(range(N_CORES)))
    LAST_RESULT = res

    out = np.empty((B, S, D), np.float32)
    bo32 = np.asarray(bo, np.float32)
    for b in range(B):
        out[b] = (res.results[2 * b]["outT"].astype(np.float32)
                  + res.results[2 * b + 1]["outT"].astype(np.float32)).T + bo32
    return out
